# revision 1
# baseline (speedup 1.0000x reference)
"""MoE MLP (Mixtral-style top-2 routing) on 8 Trainium2 NeuronCores.

Strategy: expert-parallel. The router (tiny: T x H x E) runs on host in fp32,
exactly mirroring the reference math. Tokens are grouped by expert on host;
core e runs a dense [C,H] -> silu/mul -> [C,H] MLP for expert e with f32r
(TF32-like, full PE rate) matmuls in a hand-scheduled raw-Bass program.
Host applies the top-k combine weights in a weighted scatter-add.

Device layout (per core, everything feature-on-partition, token-on-free):
  hT   [H=1024, C]   tokens for this expert, transposed
  WgT  [H, F=4096]   gate weight, transposed
  WuT  [H, F]        up weight, transposed
  WdT  [F, H]        down weight, transposed
  yT   [H, C]        output (unweighted expert output, transposed)

Loop structure: passes over tokens (<=1024 tokens resident, double-buffered);
per pass loop over 8 F-blocks of 512 (weights double-buffered); per block
loop over 512-token ct tiles. Gate/up matmuls accumulate over H in PSUM;
ScalarE applies silu into the act tile; VectorE multiplies in-place by the
up projection; down matmuls accumulate the F-block in PSUM; VectorE
accumulates y in SBUF. The PE stream runs one ct-tile ahead (gate/up of
tile n+1 issued before down of tile n) to hide the silu/mul latency.
"""

import numpy as np
import concourse.bass as bass
import concourse.mybir as mybir
from concourse.bass_utils import run_bass_kernel_spmd

f32 = mybir.dt.float32
f32r = mybir.dt.float32r

B, S, H, F, E = 4, 2048, 1024, 4096, 8
KT = H // 128  # 8 k-tiles of the H contraction
NFB = 8  # F blocks
FBLK = F // NFB  # 512
FT_PER = FBLK // 128  # 4 f-tiles per block
HT = H // 128  # 8 output H tiles
CT_W = 512  # token tile width (moving dim N)


def _split_tiles(pass_size):
    """Split a pass into ct tiles: as few tiles as possible (<=512 each),
    near-equal widths, all multiples of 128 and >= 256."""
    k = -(-pass_size // CT_W)
    base = (pass_size // k) // 128 * 128
    widths = [base] * k
    rem = (pass_size - base * k) // 128
    for i in range(rem):
        widths[i] += 128
    assert sum(widths) == pass_size and all(256 <= w <= 512 for w in widths), widths
    return widths


def build_program(pass_sizes, repeat=1, probe=None):
    """Build the per-core Bass program for the given tuple of pass sizes
    (each a multiple of 256). `repeat` re-runs the whole computation that
    many times (same I/O) — benchmarking only. `probe` builds timing
    bisection variants (wrong results)."""
    pass_sizes = list(pass_sizes)
    C = sum(pass_sizes)
    pass_tok0 = [sum(pass_sizes[:p]) for p in range(len(pass_sizes))] * repeat
    pass_sizes = pass_sizes * repeat
    NP = len(pass_sizes)
    PSMAX = max(pass_sizes)
    tiles = [_split_tiles(ps) for ps in pass_sizes]
    NCT = [len(t) for t in tiles]

    # ctg enumeration: for p, for fb, for ct -> (p, fb, ct, width, offset)
    ctg_base = [0] * (NP + 1)
    for p in range(NP):
        ctg_base[p + 1] = ctg_base[p] + NFB * NCT[p]
    TOTAL_CT = ctg_base[NP]

    ctg_pfc = []
    for p in range(NP):
        offs = [sum(tiles[p][:i]) for i in range(NCT[p])]
        for fb in range(NFB):
            for ct in range(NCT[p]):
                ctg_pfc.append((p, fb, ct, tiles[p][ct], offs[ct]))

    def ctg_end_w(w):
        p, fb = divmod(w, NFB)
        return ctg_base[p] + (fb + 1) * NCT[p]

    hc_base = [sum(NCT[:p]) for p in range(NP)]

    NW = NP * NFB

    nc = bass.Bass()
    hT = nc.declare_dram_parameter("hT", [H, C], f32r, isOutput=False)
    wg = nc.declare_dram_parameter("WgT", [H, F], f32r, isOutput=False)
    wu = nc.declare_dram_parameter("WuT", [H, F], f32r, isOutput=False)
    wd = nc.declare_dram_parameter("WdT", [F, H], f32r, isOutput=False)
    yT = nc.declare_dram_parameter("yT", [H, C], f32, isOutput=True)

    hT_v = hT.rearrange("(k p) t -> p k t", p=128)  # [128, KT, C]
    wg_v = wg.rearrange("(k p) f -> p k f", p=128)  # [128, KT, F]
    wu_v = wu.rearrange("(k p) f -> p k f", p=128)
    wd_v = wd.rearrange("(q p) h -> p q h", p=128)  # [128, F//128, H]
    yT_v = yT.rearrange("(k p) t -> p k t", p=128)  # [128, HT, C]

    from contextlib import ExitStack

    with ExitStack() as ctx:
        en = ctx.enter_context
        h_sb = en(nc.sbuf_tensor("h_sb", [128, KT, PSMAX], f32r))
        h_pre = en(nc.sbuf_tensor("h_pre", [128, KT, CT_W], f32r))
        y_sb = en(nc.sbuf_tensor("y_sb", [128, HT, PSMAX], f32))
        wg_sb = en(nc.sbuf_tensor("wg_sb", [128, 2, KT, FBLK], f32r))
        wu_sb = en(nc.sbuf_tensor("wu_sb", [128, 2, KT, FBLK], f32r))
        wd_sb = en(nc.sbuf_tensor("wd_sb", [128, 2, FT_PER, H], f32r))
        act_sb = en(nc.sbuf_tensor("act_sb", [128, 2, FT_PER, CT_W], f32r))

        g_ps = [en(nc.psum_tensor(f"g_ps{i}", [128, CT_W], f32)) for i in range(2)]
        u_ps = [en(nc.psum_tensor(f"u_ps{i}", [128, CT_W], f32)) for i in range(2)]
        yp_ps = [en(nc.psum_tensor(f"yp_ps{i}", [128, CT_W], f32)) for i in range(4)]

        s_w = en(nc.semaphore())  # weight DMAs done (16/dma, 48/block)
        s_h = en(nc.semaphore())  # hT pass loads (16/pass)
        s_g = en(nc.semaphore())  # PE: gate groups done (1/gi)
        s_u = en(nc.semaphore())  # PE: up groups done (1/gi)
        s_silu = en(nc.semaphore())  # ACT: silu into act done (1/gi)
        s_mul = en(nc.semaphore())  # DVE: act *= up done (1/gi)
        s_down = en(nc.semaphore())  # PE: down groups done (1/di)
        s_yupd = en(nc.semaphore())  # DVE: y accum done (1/di)
        s_ydma = en(nc.semaphore())  # y store DMAs done (16/pass)

        block = en(nc.Block())

        # ---------------- weight DMA stream (sync engine / HWDGE) --------
        # s_w thresholds: W=0 is split into ft-granular pieces so the PE can
        # start after the first ft column lands.
        # block 0 issues 9 DMAs (8 ft-granular wg/wu + wd) = 144 counts;
        # blocks >= 1 issue 3 DMAs (48 counts) each, order wg, wu, wd.
        def sw_need_gu(w, ft):
            if w == 0:
                return 32 * (ft + 1)
            return 144 + 48 * (w - 1) + 32

        def sw_need_down(w):
            return 144 + 48 * w

        @block.sync
        def _(sync):
            for w in range(2 if probe == "wonce" else NW):
                p, fb = divmod(w, NFB)
                buf = w % 2
                if w >= 2:
                    if probe == "nodown":
                        sync.wait_ge(s_u, 4 * ctg_end_w(w - 2))
                    else:
                        sync.wait_ge(s_down, 8 * ctg_end_w(w - 2))
                fsl = slice(fb * FBLK, (fb + 1) * FBLK)
                qsl = slice(fb * FT_PER, (fb + 1) * FT_PER)
                if w == 0:
                    for ft in range(FT_PER):
                        f0 = fb * FBLK + ft * 128
                        sync.dma_start(
                            wg_sb[:, buf, :, ft * 128 : (ft + 1) * 128],
                            wg_v[:, :, f0 : f0 + 128],
                        ).then_inc(s_w, 16)
                        sync.dma_start(
                            wu_sb[:, buf, :, ft * 128 : (ft + 1) * 128],
                            wu_v[:, :, f0 : f0 + 128],
                        ).then_inc(s_w, 16)
                else:
                    sync.dma_start(wg_sb[:, buf], wg_v[:, :, fsl]).then_inc(s_w, 16)
                    sync.dma_start(wu_sb[:, buf], wu_v[:, :, fsl]).then_inc(s_w, 16)
                sync.dma_start(wd_sb[:, buf], wd_v[:, qsl, :]).then_inc(s_w, 16)

        # ---------------- hT loads + y stores (gpsimd / SWDGE) -----------
        @block.gpsimd
        def _(gp):
            def load_h(p):
                # chunk 0 of pass p>=1 goes to the h_pre prefetch buffer,
                # issued as soon as the previous pass's first gu released it
                if p >= 1:
                    # h_pre is read at ct==0 of EVERY fb of pass p-1; free
                    # only after the last fb's gu of pass p-1
                    gp.wait_ge(s_u, 4 * (ctg_base[p - 1] + 7 * NCT[p - 1] + 1))
                    w0 = tiles[p][0]
                    tsl = slice(pass_tok0[p], pass_tok0[p] + w0)
                    gp.dma_start(h_pre[:, :, :w0], hT_v[:, :, tsl]).then_inc(s_h, 16)
                    gp.wait_ge(s_u, 4 * ctg_base[p])
                off = 0
                for i, wdt in enumerate(tiles[p]):
                    if p >= 1 and i == 0:
                        off += wdt
                        continue
                    tsl = slice(pass_tok0[p] + off, pass_tok0[p] + off + wdt)
                    gp.dma_start(
                        h_sb[:, :, off : off + wdt], hT_v[:, :, tsl]
                    ).then_inc(s_h, 16)
                    off += wdt

            def store_y(p):
                if probe == "nodown":
                    gp.wait_ge(s_mul, 4 * ctg_base[p + 1])
                elif probe in ("noyupd", "nosilu", "peonly"):
                    gp.wait_ge(s_down, 8 * ctg_base[p + 1])
                else:
                    gp.wait_ge(s_yupd, 8 * ctg_base[p + 1])
                tsl = slice(pass_tok0[p], pass_tok0[p] + pass_sizes[p])
                gp.dma_start(yT_v[:, :, tsl], y_sb[:, :, : pass_sizes[p]]).then_inc(
                    s_ydma, 16
                )

            if probe == "peonly":
                # init act with finite values (f32r memset fails ISA check)
                for b in range(2):
                    for ft in range(FT_PER):
                        gp.dma_start(act_sb[:, b, ft, :], hT_v[:, ft, 0:CT_W]).then_inc(
                            s_mul, 16
                        )
            load_h(0)
            for p in range(1, NP):
                load_h(p)
                if probe in ("nodown", "noyupd", "nosilu", "peonly"):
                    store_y(p - 1)
            if probe in ("nodown", "noyupd", "nosilu", "peonly"):
                store_y(NP - 1)

        # ---------------- PE stream (one ct-tile lookahead) ----------------
        @block.tensor
        def _(te):
            def gu(ctg):
                p, fb, ct, ctw, coff = ctg_pfc[ctg]
                w = p * NFB + fb
                buf = w % 2
                if fb == 0:
                    te.wait_ge(s_h, 16 * (hc_base[p] + ct + 1))
                if ct == 0 and w > 0:
                    te.wait_ge(
                        s_w,
                        min(sw_need_gu(w, 0), 96) if probe == "wonce" else sw_need_gu(w, 0),
                    )
                use_pre = p >= 1 and ct == 0
                csl = slice(coff, coff + ctw)
                for ft in range(FT_PER):
                    gi = ctg * 4 + ft
                    gb = gi % 2
                    if w == 0 and ct == 0 and probe != "wonce":
                        te.wait_ge(s_w, sw_need_gu(0, ft))
                    elif w == 0 and ct == 0 and ft == 0:
                        te.wait_ge(s_w, 64)
                    if gi >= 2 and probe not in ("nosilu", "peonly"):
                        te.wait_ge(s_silu, gi - 1)
                    for k in range(KT):
                        rhs = h_pre[:, k, :ctw] if use_pre else h_sb[:, k, csl]
                        mm = nc.tensor.matmul(
                            g_ps[gb][:, :ctw],
                            wg_sb[:, buf, k, ft * 128 : (ft + 1) * 128],
                            rhs,
                            start=(k == 0),
                            stop=(k == KT - 1),
                        )
                        if k == KT - 1:
                            mm.then_inc(s_g, 1)
                    if gi >= 2 and probe not in ("nosilu", "peonly"):
                        te.wait_ge(s_mul, gi - 1)
                    for k in range(KT):
                        rhs = h_pre[:, k, :ctw] if use_pre else h_sb[:, k, csl]
                        mm = nc.tensor.matmul(
                            u_ps[gb][:, :ctw],
                            wu_sb[:, buf, k, ft * 128 : (ft + 1) * 128],
                            rhs,
                            start=(k == 0),
                            stop=(k == KT - 1),
                        )
                        if k == KT - 1:
                            mm.then_inc(s_u, 1)

            def down(ctg):
                p, fb, ct, ctw, coff = ctg_pfc[ctg]
                ab = ctg % 2
                if ct == 0:
                    w = p * NFB + fb
                    te.wait_ge(
                        s_w,
                        min(sw_need_down(w), 144) if probe == "wonce" else sw_need_down(w),
                    )
                if probe == "peonly":
                    if ctg == 0:
                        te.wait_ge(s_mul, 128)  # act_sb init done
                elif probe != "nosilu":
                    te.wait_ge(s_mul, 4 * (ctg + 1))
                w = p * NFB + fb
                buf = w % 2
                for ht in range(HT):
                    di = ctg * 8 + ht
                    db = di % 4
                    if di >= 4 and probe not in ("noyupd", "nosilu", "peonly"):
                        te.wait_ge(s_yupd, di - 3)
                    for ft in range(FT_PER):
                        mm = nc.tensor.matmul(
                            yp_ps[db][:, :ctw],
                            wd_sb[:, buf, ft, ht * 128 : (ht + 1) * 128],
                            act_sb[:, ab, ft, :ctw],
                            start=(ft == 0),
                            stop=(ft == FT_PER - 1),
                        )
                        if ft == FT_PER - 1:
                            mm.then_inc(s_down, 1)

            gu(0)
            for ctg in range(TOTAL_CT):
                if ctg + 1 < TOTAL_CT:
                    same_pass = ctg_pfc[ctg + 1][0] == ctg_pfc[ctg][0]
                    if same_pass:
                        gu(ctg + 1)
                        if probe != "nodown":
                            down(ctg)
                    else:
                        if probe != "nodown":
                            down(ctg)
                        gu(ctg + 1)
                elif probe != "nodown":
                    down(ctg)

        # ---------------- ACT stream (silu into act tile) ------------------
        @block.scalar
        def _(sc):
            if probe == "peonly":
                sc.nop()
                return
            if probe == "nosilu":
                return

            def sc_store_y(p):
                sc.wait_ge(s_yupd, 8 * ctg_base[p + 1])
                tsl = slice(pass_tok0[p], pass_tok0[p] + pass_sizes[p])
                sc.dma_start(yT_v[:, :, tsl], y_sb[:, :, : pass_sizes[p]]).then_inc(
                    s_ydma, 16
                )

            for ctg in range(TOTAL_CT):
                p = ctg_pfc[ctg][0]
                if ctg > 0 and ctg_pfc[ctg - 1][0] != p:
                    sc_store_y(p - 1)
                ab = ctg % 2
                ctw = ctg_pfc[ctg][3]
                for ft in range(FT_PER):
                    gi = ctg * 4 + ft
                    gb = gi % 2
                    if ft == 0 and ctg >= 2:
                        # WAR on act_sb[ab]: down mms of ctg-2 done
                        if probe == "nodown":
                            sc.wait_ge(s_mul, 4 * (ctg - 1))
                        else:
                            sc.wait_ge(s_down, 8 * (ctg - 1))
                    sc.wait_ge(s_g, gi + 1)
                    nc.scalar.activation(
                        act_sb[:, ab, ft, :ctw],
                        g_ps[gb][:, :ctw],
                        mybir.ActivationFunctionType.Silu,
                    ).then_inc(s_silu, 1)
            sc_store_y(NP - 1)

        # ---------------- DVE stream (mul + y accumulate) ------------------
        @block.vector
        def _(ve):
            if probe in ("nosilu", "peonly"):
                return

            def muls(ctg):
                ab = ctg % 2
                ctw = ctg_pfc[ctg][3]
                for ft in range(FT_PER):
                    gi = ctg * 4 + ft
                    gb = gi % 2
                    ve.wait_ge(s_silu, gi + 1)
                    ve.wait_ge(s_u, gi + 1)
                    nc.vector.tensor_mul(
                        act_sb[:, ab, ft, :ctw],
                        act_sb[:, ab, ft, :ctw],
                        u_ps[gb][:, :ctw],
                    ).then_inc(s_mul, 1)

            def yupd(ctg):
                if probe in ("nodown", "noyupd"):
                    return
                p, fb, ct, ctw, coff = ctg_pfc[ctg]
                csl = slice(coff, coff + ctw)
                for ht in range(HT):
                    di = ctg * 8 + ht
                    db = di % 4
                    ve.wait_ge(s_down, di + 1)
                    if fb == 0 and ct == 0 and ht == 0 and p > 0:
                        ve.wait_ge(s_ydma, 16 * p)
                    if fb == 0:
                        nc.vector.tensor_copy(
                            y_sb[:, ht, csl], yp_ps[db][:, :ctw]
                        ).then_inc(s_yupd, 1)
                    else:
                        nc.vector.tensor_add(
                            y_sb[:, ht, csl], y_sb[:, ht, csl], yp_ps[db][:, :ctw]
                        ).then_inc(s_yupd, 1)

            muls(0)
            for ctg in range(TOTAL_CT):
                # mirror the PE stream's emission order exactly, else the
                # crossing steps (down before gu) deadlock against us
                if ctg + 1 < TOTAL_CT:
                    same_pass = ctg_pfc[ctg + 1][0] == ctg_pfc[ctg][0]
                    if same_pass:
                        muls(ctg + 1)
                        yupd(ctg)
                    else:
                        yupd(ctg)
                        muls(ctg + 1)
                else:
                    yupd(ctg)

    return nc


# ----------------------------------------------------------------------------
# Host side
# ----------------------------------------------------------------------------


def _route(h, Wr, topk):
    """Exact fp32 replica of the reference router. Returns sel [T,k], w [T,k]."""
    logits = h @ Wr.T  # [T, E]
    logits = logits.astype(np.float32)
    m = logits.max(axis=-1, keepdims=True)
    e = np.exp(logits - m)
    p = e / e.sum(axis=-1, keepdims=True)
    sel = np.argsort(-p, axis=-1, kind="stable")[:, :topk]  # ties -> lower idx
    w = np.take_along_axis(p, sel, axis=-1)
    if topk != 1:
        w = w / w.sum(axis=-1, keepdims=True)
    return sel, w.astype(np.float32)


def _pass_sizes(C):
    n = -(-C // 1152)  # keep h_sb + h_pre + y_sb within SBUF
    base = (C // n) // 128 * 128
    out = [base] * n
    rem = (C - base * n) // 128
    for i in range(rem):
        out[i] += 128
    assert sum(out) == C and all(ps <= 1152 for ps in out)
    return tuple(out)


def kernel(x, Wr, Wg, Wu, Wd, topk):
    topk = int(topk)
    x = np.asarray(x, dtype=np.float32)
    Wr = np.asarray(Wr, dtype=np.float32)
    Wg = np.asarray(Wg, dtype=np.float32)
    Wu = np.asarray(Wu, dtype=np.float32)
    Wd = np.asarray(Wd, dtype=np.float32)

    T = x.shape[0] * x.shape[1]
    h = np.ascontiguousarray(x.reshape(T, H))

    sel, w = _route(h, Wr, topk)

    idx = [None] * E
    wts = [None] * E
    for e in range(E):
        tok, kk = np.nonzero(sel == e)
        idx[e] = tok
        wts[e] = w[tok, kk]
    counts = [len(i) for i in idx]
    maxc = max(max(counts), 1)
    C = max(512, ((maxc + 255) // 256) * 256)

    nc = build_program(_pass_sizes(C))

    hTfull = h.T  # [H, T] view
    in_maps = []
    for e in range(E):
        cnt = counts[e]
        hTe = np.zeros((H, C), dtype=np.float32)
        if cnt:
            hTe[:, :cnt] = hTfull[:, idx[e]]
        in_maps.append(
            {
                "hT": hTe,
                "WgT": np.ascontiguousarray(Wg[e].T),  # [H, F]
                "WuT": np.ascontiguousarray(Wu[e].T),  # [H, F]
                "WdT": np.ascontiguousarray(Wd[e].T),  # [F, H]
            }
        )

    res = run_bass_kernel_spmd(nc, in_maps, core_ids=list(range(E)))

    out = np.zeros((T, H), dtype=np.float32)
    for e in range(E):
        cnt = counts[e]
        if cnt:
            ye = res.results[e]["yT"][:, :cnt].T  # [cnt, H]
            out[idx[e]] += wts[e][:, None] * ye
    return out.reshape(x.shape)



# revision 8
# speedup vs baseline: 1.1258x; 1.1258x over previous
"""MoE MLP (Mixtral-style top-2 routing) on 8 Trainium2 NeuronCores.

Strategy: expert-parallel. The router (tiny: T x H x E) runs on host in fp32,
exactly mirroring the reference math. Tokens are grouped by expert on host;
core e runs a dense [C,H] -> silu/mul -> [C,H] MLP for expert e with bf16
matmuls (full PE rate + fast weight load) in a hand-scheduled raw-Bass
program. Host applies the top-k combine weights in a weighted scatter-add.

Device layout (per core, everything feature-on-partition, token-on-free):
  hT   [H=1024, C]   tokens for this expert, transposed (bf16)
  WgT  [H, F=4096]   gate weight, transposed (bf16)
  WuT  [H, F]        up weight, transposed (bf16)
  WdT  [F, H]        down weight, transposed (bf16)
  yT   [H, C]        output (unweighted expert output, transposed, f32)

Loop structure: passes over tokens (<=2816 tokens resident; a single pass in
practice); per pass loop over 8 F-blocks of 512 (weights double-buffered);
per block loop over 512-token ct tiles. Gate/up matmuls accumulate over H in
PSUM; ScalarE applies silu into the act tile; VectorE multiplies in-place by
the up projection; down matmuls accumulate the F-block in PSUM; VectorE
accumulates y in SBUF. The PE stream runs one ct-tile ahead (gate/up of
tile n+1 issued before down of tile n) to hide the silu/mul latency. y is
stored per-ht-tile as the last F-block's accumulations finish, so the store
overlaps the tail of compute.
"""

import ml_dtypes
import numpy as np
import concourse.bass as bass
import concourse.mybir as mybir
from concourse.bass_utils import run_bass_kernel_spmd

f32 = mybir.dt.float32
bf16 = mybir.dt.bfloat16
np_bf16 = ml_dtypes.bfloat16

B, S, H, F, E = 4, 2048, 1024, 4096, 8
KT = H // 128  # 8 k-tiles of the H contraction
NFB = 8  # F blocks
FBLK = F // NFB  # 512
FT_PER = FBLK // 128  # 4 f-tiles per block
HT = H // 128  # 8 output H tiles
CT_W = 512  # token tile width (moving dim N)


def _split_tiles(pass_size):
    """Split a pass into ct tiles: as few tiles as possible (<=512 each),
    near-equal widths, all multiples of 128 and >= 256."""
    k = -(-pass_size // CT_W)
    base = (pass_size // k) // 128 * 128
    widths = [base] * k
    rem = (pass_size - base * k) // 128
    for i in range(rem):
        widths[i] += 128
    assert sum(widths) == pass_size and all(256 <= w <= 512 for w in widths), widths
    return widths


def build_program(pass_sizes, repeat=1, probe=None):
    """Build the per-core Bass program for the given tuple of pass sizes
    (each a multiple of 256). `repeat` re-runs the whole computation that
    many times (same I/O) — benchmarking only. `probe` builds timing
    bisection variants (wrong results)."""
    pass_sizes = list(pass_sizes)
    C = sum(pass_sizes)
    pass_tok0 = [sum(pass_sizes[:p]) for p in range(len(pass_sizes))] * repeat
    pass_sizes = pass_sizes * repeat
    NP = len(pass_sizes)
    PSMAX = max(pass_sizes)
    tiles = [_split_tiles(ps) for ps in pass_sizes]
    NCT = [len(t) for t in tiles]

    # ctg enumeration: for p, for fb, for ct -> (p, fb, ct, width, offset)
    ctg_base = [0] * (NP + 1)
    for p in range(NP):
        ctg_base[p + 1] = ctg_base[p] + NFB * NCT[p]
    TOTAL_CT = ctg_base[NP]

    ctg_pfc = []
    for p in range(NP):
        offs = [sum(tiles[p][:i]) for i in range(NCT[p])]
        for fb in range(NFB):
            for ct in range(NCT[p]):
                ctg_pfc.append((p, fb, ct, tiles[p][ct], offs[ct]))

    def ctg_end_w(w):
        p, fb = divmod(w, NFB)
        return ctg_base[p] + (fb + 1) * NCT[p]

    hc_base = [sum(NCT[:p]) for p in range(NP)]

    NW = NP * NFB

    nc = bass.Bass()
    hT = nc.declare_dram_parameter("hT", [H, C], bf16, isOutput=False)
    wg = nc.declare_dram_parameter("WgT", [H, F], bf16, isOutput=False)
    wu = nc.declare_dram_parameter("WuT", [H, F], bf16, isOutput=False)
    wd = nc.declare_dram_parameter("WdT", [F, H], bf16, isOutput=False)
    yT = nc.declare_dram_parameter("yT", [H, C], f32, isOutput=True)

    hT_v = hT.rearrange("(k p) t -> p k t", p=128)  # [128, KT, C]
    wg_v = wg.rearrange("(k p) f -> p k f", p=128)  # [128, KT, F]
    wu_v = wu.rearrange("(k p) f -> p k f", p=128)
    wd_v = wd.rearrange("(q p) h -> p q h", p=128)  # [128, F//128, H]
    yT_v = yT.rearrange("(k p) t -> p k t", p=128)  # [128, HT, C]

    from contextlib import ExitStack

    with ExitStack() as ctx:
        en = ctx.enter_context
        h_sb = en(nc.sbuf_tensor("h_sb", [128, KT, PSMAX], bf16))
        h_pre = en(nc.sbuf_tensor("h_pre", [128, KT, CT_W], bf16))
        y_sb = en(nc.sbuf_tensor("y_sb", [128, HT, PSMAX], f32))
        wg_sb = en(nc.sbuf_tensor("wg_sb", [128, 2, KT, FBLK], bf16))
        wu_sb = en(nc.sbuf_tensor("wu_sb", [128, 2, KT, FBLK], bf16))
        wd_sb = en(nc.sbuf_tensor("wd_sb", [128, 2, FT_PER, H], bf16))
        act_sb = en(nc.sbuf_tensor("act_sb", [128, 2, FT_PER, CT_W], bf16))

        g_ps = [en(nc.psum_tensor(f"g_ps{i}", [128, CT_W], f32)) for i in range(2)]
        u_ps = [en(nc.psum_tensor(f"u_ps{i}", [128, CT_W], f32)) for i in range(2)]
        yp_ps = [en(nc.psum_tensor(f"yp_ps{i}", [128, CT_W], f32)) for i in range(4)]

        s_w = en(nc.semaphore())  # weight DMAs done (16/dma, 48/block)
        s_h = en(nc.semaphore())  # hT pass loads (16/pass)
        s_g = en(nc.semaphore())  # PE: gate groups done (1/gi)
        s_u = en(nc.semaphore())  # PE: up groups done (1/gi)
        s_silu = en(nc.semaphore())  # ACT: silu into act done (1/gi)
        s_mul = en(nc.semaphore())  # DVE: act *= up done (1/gi)
        s_down = en(nc.semaphore())  # PE: down groups done (1/di)
        s_yupd = en(nc.semaphore())  # DVE: y accum done (1/di)
        s_ydma = en(nc.semaphore())  # y store DMAs done (16/pass)

        block = en(nc.Block())

        # ---------------- weight DMA stream (sync engine / HWDGE) --------
        # s_w thresholds: W=0 is split into ft-granular pieces so the PE can
        # start after the first ft column lands.
        # block 0 issues 9 DMAs (8 ft-granular wg/wu + wd) = 144 counts;
        # blocks >= 1 issue 3 DMAs (48 counts) each, order wg, wu, wd.
        def sw_need_gu(w, ft):
            if w == 0:
                return 32 * (ft + 1)
            return 144 + 48 * (w - 1) + 32

        def sw_need_down(w):
            return 144 + 48 * w

        @block.sync
        def _(sync):
            for w in range(2 if probe == "wonce" else NW):
                p, fb = divmod(w, NFB)
                buf = w % 2
                if w >= 2:
                    if probe == "nodown":
                        sync.wait_ge(s_u, 4 * ctg_end_w(w - 2))
                    else:
                        sync.wait_ge(s_down, 8 * ctg_end_w(w - 2))
                fsl = slice(fb * FBLK, (fb + 1) * FBLK)
                qsl = slice(fb * FT_PER, (fb + 1) * FT_PER)
                if w == 0:
                    for ft in range(FT_PER):
                        f0 = fb * FBLK + ft * 128
                        sync.dma_start(
                            wg_sb[:, buf, :, ft * 128 : (ft + 1) * 128],
                            wg_v[:, :, f0 : f0 + 128],
                        ).then_inc(s_w, 16)
                        sync.dma_start(
                            wu_sb[:, buf, :, ft * 128 : (ft + 1) * 128],
                            wu_v[:, :, f0 : f0 + 128],
                        ).then_inc(s_w, 16)
                else:
                    sync.dma_start(wg_sb[:, buf], wg_v[:, :, fsl]).then_inc(s_w, 16)
                    sync.dma_start(wu_sb[:, buf], wu_v[:, :, fsl]).then_inc(s_w, 16)
                sync.dma_start(wd_sb[:, buf], wd_v[:, qsl, :]).then_inc(s_w, 16)

        # ---------------- hT loads + y stores (gpsimd / SWDGE) -----------
        @block.gpsimd
        def _(gp):
            def load_h(p):
                # chunk 0 of pass p>=1 goes to the h_pre prefetch buffer,
                # issued as soon as the previous pass's first gu released it
                if p >= 1:
                    # h_pre is read at ct==0 of EVERY fb of pass p-1; free
                    # only after the last fb's gu of pass p-1
                    gp.wait_ge(s_u, 4 * (ctg_base[p - 1] + 7 * NCT[p - 1] + 1))
                    w0 = tiles[p][0]
                    tsl = slice(pass_tok0[p], pass_tok0[p] + w0)
                    gp.dma_start(h_pre[:, :, :w0], hT_v[:, :, tsl]).then_inc(s_h, 16)
                    gp.wait_ge(s_u, 4 * ctg_base[p])
                off = 0
                for i, wdt in enumerate(tiles[p]):
                    if p >= 1 and i == 0:
                        off += wdt
                        continue
                    tsl = slice(pass_tok0[p] + off, pass_tok0[p] + off + wdt)
                    gp.dma_start(
                        h_sb[:, :, off : off + wdt], hT_v[:, :, tsl]
                    ).then_inc(s_h, 16)
                    off += wdt

            def store_y(p):
                if probe == "nodown":
                    gp.wait_ge(s_mul, 4 * ctg_base[p + 1])
                elif probe in ("noyupd", "nosilu", "peonly"):
                    gp.wait_ge(s_down, 8 * ctg_base[p + 1])
                else:
                    gp.wait_ge(s_yupd, 8 * ctg_base[p + 1])
                tsl = slice(pass_tok0[p], pass_tok0[p] + pass_sizes[p])
                gp.dma_start(yT_v[:, :, tsl], y_sb[:, :, : pass_sizes[p]]).then_inc(
                    s_ydma, 16
                )

            if probe == "peonly":
                # init act with finite values (f32r memset fails ISA check)
                for b in range(2):
                    for ft in range(FT_PER):
                        gp.dma_start(act_sb[:, b, ft, :], hT_v[:, ft, 0:CT_W]).then_inc(
                            s_mul, 16
                        )
            load_h(0)
            for p in range(1, NP):
                load_h(p)
                if probe in ("nodown", "noyupd", "nosilu", "peonly"):
                    store_y(p - 1)
            if probe in ("nodown", "noyupd", "nosilu", "peonly"):
                store_y(NP - 1)

        # ---------------- PE stream (one ct-tile lookahead) ----------------
        @block.tensor
        def _(te):
            def gu(ctg):
                p, fb, ct, ctw, coff = ctg_pfc[ctg]
                w = p * NFB + fb
                buf = w % 2
                if fb == 0:
                    te.wait_ge(s_h, 16 * (hc_base[p] + ct + 1))
                if ct == 0 and w > 0:
                    te.wait_ge(
                        s_w,
                        min(sw_need_gu(w, 0), 96) if probe == "wonce" else sw_need_gu(w, 0),
                    )
                use_pre = p >= 1 and ct == 0
                csl = slice(coff, coff + ctw)
                for ft in range(FT_PER):
                    gi = ctg * 4 + ft
                    gb = gi % 2
                    if w == 0 and ct == 0 and probe != "wonce":
                        te.wait_ge(s_w, sw_need_gu(0, ft))
                    elif w == 0 and ct == 0 and ft == 0:
                        te.wait_ge(s_w, 64)
                    if gi >= 2 and probe not in ("nosilu", "peonly"):
                        te.wait_ge(s_silu, gi - 1)
                    for k in range(KT):
                        rhs = h_pre[:, k, :ctw] if use_pre else h_sb[:, k, csl]
                        mm = nc.tensor.matmul(
                            g_ps[gb][:, :ctw],
                            wg_sb[:, buf, k, ft * 128 : (ft + 1) * 128],
                            rhs,
                            start=(k == 0),
                            stop=(k == KT - 1),
                        )
                        if k == KT - 1:
                            mm.then_inc(s_g, 1)
                    if gi >= 2 and probe not in ("nosilu", "peonly"):
                        te.wait_ge(s_mul, gi - 1)
                    for k in range(KT):
                        rhs = h_pre[:, k, :ctw] if use_pre else h_sb[:, k, csl]
                        mm = nc.tensor.matmul(
                            u_ps[gb][:, :ctw],
                            wu_sb[:, buf, k, ft * 128 : (ft + 1) * 128],
                            rhs,
                            start=(k == 0),
                            stop=(k == KT - 1),
                        )
                        if k == KT - 1:
                            mm.then_inc(s_u, 1)

            def down(ctg):
                p, fb, ct, ctw, coff = ctg_pfc[ctg]
                ab = ctg % 2
                if ct == 0:
                    w = p * NFB + fb
                    te.wait_ge(
                        s_w,
                        min(sw_need_down(w), 144) if probe == "wonce" else sw_need_down(w),
                    )
                if probe == "peonly":
                    if ctg == 0:
                        te.wait_ge(s_mul, 128)  # act_sb init done
                elif probe != "nosilu":
                    te.wait_ge(s_mul, 4 * (ctg + 1))
                w = p * NFB + fb
                buf = w % 2
                for ht in range(HT):
                    di = ctg * 8 + ht
                    db = di % 4
                    if di >= 4 and probe not in ("noyupd", "nosilu", "peonly"):
                        te.wait_ge(s_yupd, di - 3)
                    for ft in range(FT_PER):
                        mm = nc.tensor.matmul(
                            yp_ps[db][:, :ctw],
                            wd_sb[:, buf, ft, ht * 128 : (ht + 1) * 128],
                            act_sb[:, ab, ft, :ctw],
                            start=(ft == 0),
                            stop=(ft == FT_PER - 1),
                        )
                        if ft == FT_PER - 1:
                            mm.then_inc(s_down, 1)

            gu(0)
            for ctg in range(TOTAL_CT):
                if ctg + 1 < TOTAL_CT:
                    same_pass = ctg_pfc[ctg + 1][0] == ctg_pfc[ctg][0]
                    if same_pass:
                        gu(ctg + 1)
                        if probe != "nodown":
                            down(ctg)
                    else:
                        if probe != "nodown":
                            down(ctg)
                        gu(ctg + 1)
                elif probe != "nodown":
                    down(ctg)

        # ---------------- ACT stream (silu into act tile) ------------------
        @block.scalar
        def _(sc):
            if probe == "peonly":
                sc.nop()
                return
            if probe == "nosilu":
                return

            def sc_store_y(p):
                # Chunked per-(ct, ht) stores: each chunk is final as soon as
                # the last F-block's yupd for it lands, so stores overlap the
                # tail of compute instead of waiting for the whole pass.
                offs = [sum(tiles[p][:i]) for i in range(NCT[p])]
                for ct in range(NCT[p]):
                    ctg = ctg_base[p + 1] - NCT[p] + ct
                    coff, ctw = offs[ct], tiles[p][ct]
                    tsl = slice(pass_tok0[p] + coff, pass_tok0[p] + coff + ctw)
                    for ht in range(HT):
                        sc.wait_ge(s_yupd, 8 * ctg + ht + 1)
                        sc.dma_start(
                            yT_v[:, ht, tsl], y_sb[:, ht, coff : coff + ctw]
                        ).then_inc(s_ydma, 16)

            for ctg in range(TOTAL_CT):
                p = ctg_pfc[ctg][0]
                if ctg > 0 and ctg_pfc[ctg - 1][0] != p:
                    sc_store_y(p - 1)
                ab = ctg % 2
                ctw = ctg_pfc[ctg][3]
                for ft in range(FT_PER):
                    gi = ctg * 4 + ft
                    gb = gi % 2
                    if ft == 0 and ctg >= 2:
                        # WAR on act_sb[ab]: down mms of ctg-2 done
                        if probe == "nodown":
                            sc.wait_ge(s_mul, 4 * (ctg - 1))
                        else:
                            sc.wait_ge(s_down, 8 * (ctg - 1))
                    sc.wait_ge(s_g, gi + 1)
                    nc.scalar.activation(
                        act_sb[:, ab, ft, :ctw],
                        g_ps[gb][:, :ctw],
                        mybir.ActivationFunctionType.Silu,
                    ).then_inc(s_silu, 1)
            sc_store_y(NP - 1)

        # ---------------- DVE stream (mul + y accumulate) ------------------
        @block.vector
        def _(ve):
            if probe in ("nosilu", "peonly"):
                return

            def muls(ctg):
                ab = ctg % 2
                ctw = ctg_pfc[ctg][3]
                for ft in range(FT_PER):
                    gi = ctg * 4 + ft
                    gb = gi % 2
                    ve.wait_ge(s_silu, gi + 1)
                    ve.wait_ge(s_u, gi + 1)
                    nc.vector.tensor_mul(
                        act_sb[:, ab, ft, :ctw],
                        act_sb[:, ab, ft, :ctw],
                        u_ps[gb][:, :ctw],
                    ).then_inc(s_mul, 1)

            def yupd(ctg):
                if probe in ("nodown", "noyupd"):
                    return
                p, fb, ct, ctw, coff = ctg_pfc[ctg]
                csl = slice(coff, coff + ctw)
                for ht in range(HT):
                    di = ctg * 8 + ht
                    db = di % 4
                    ve.wait_ge(s_down, di + 1)
                    if fb == 0 and ct == 0 and ht == 0 and p > 0:
                        # all of the previous pass's chunked y stores done
                        ve.wait_ge(s_ydma, 16 * 8 * hc_base[p])
                    if fb == 0:
                        nc.vector.tensor_copy(
                            y_sb[:, ht, csl], yp_ps[db][:, :ctw]
                        ).then_inc(s_yupd, 1)
                    else:
                        nc.vector.tensor_add(
                            y_sb[:, ht, csl], y_sb[:, ht, csl], yp_ps[db][:, :ctw]
                        ).then_inc(s_yupd, 1)

            muls(0)
            for ctg in range(TOTAL_CT):
                # mirror the PE stream's emission order exactly, else the
                # crossing steps (down before gu) deadlock against us
                if ctg + 1 < TOTAL_CT:
                    same_pass = ctg_pfc[ctg + 1][0] == ctg_pfc[ctg][0]
                    if same_pass:
                        muls(ctg + 1)
                        yupd(ctg)
                    else:
                        yupd(ctg)
                        muls(ctg + 1)
                else:
                    yupd(ctg)

    return nc


# ----------------------------------------------------------------------------
# Host side
# ----------------------------------------------------------------------------


def _route(h, Wr, topk):
    """Exact fp32 replica of the reference router. Returns sel [T,k], w [T,k]."""
    logits = h @ Wr.T  # [T, E]
    logits = logits.astype(np.float32)
    m = logits.max(axis=-1, keepdims=True)
    e = np.exp(logits - m)
    p = e / e.sum(axis=-1, keepdims=True)
    sel = np.argsort(-p, axis=-1, kind="stable")[:, :topk]  # ties -> lower idx
    w = np.take_along_axis(p, sel, axis=-1)
    if topk != 1:
        w = w / w.sum(axis=-1, keepdims=True)
    return sel, w.astype(np.float32)


def _pass_sizes(C):
    # bf16 h + f32 y: 48 B/token/partition; 2816 tokens + weights fit SBUF
    n = -(-C // 2816)
    base = (C // n) // 128 * 128
    out = [base] * n
    rem = (C - base * n) // 128
    for i in range(rem):
        out[i] += 128
    assert sum(out) == C and all(ps <= 2816 for ps in out)
    return tuple(out)


def kernel(x, Wr, Wg, Wu, Wd, topk):
    topk = int(topk)
    x = np.asarray(x, dtype=np.float32)
    Wr = np.asarray(Wr, dtype=np.float32)
    Wg = np.asarray(Wg, dtype=np.float32)
    Wu = np.asarray(Wu, dtype=np.float32)
    Wd = np.asarray(Wd, dtype=np.float32)

    T = x.shape[0] * x.shape[1]
    h = np.ascontiguousarray(x.reshape(T, H))

    sel, w = _route(h, Wr, topk)

    idx = [None] * E
    wts = [None] * E
    for e in range(E):
        tok, kk = np.nonzero(sel == e)
        idx[e] = tok
        wts[e] = w[tok, kk]
    counts = [len(i) for i in idx]
    maxc = max(max(counts), 1)
    C = max(512, ((maxc + 127) // 128) * 128)

    nc = build_program(_pass_sizes(C))

    h16 = h.astype(np_bf16)
    hTfull = h16.T  # [H, T] view
    in_maps = []
    for e in range(E):
        cnt = counts[e]
        hTe = np.zeros((H, C), dtype=np_bf16)
        if cnt:
            hTe[:, :cnt] = hTfull[:, idx[e]]
        in_maps.append(
            {
                "hT": hTe,
                "WgT": np.ascontiguousarray(Wg[e].astype(np_bf16).T),  # [H, F]
                "WuT": np.ascontiguousarray(Wu[e].astype(np_bf16).T),  # [H, F]
                "WdT": np.ascontiguousarray(Wd[e].astype(np_bf16).T),  # [F, H]
            }
        )

    res = run_bass_kernel_spmd(nc, in_maps, core_ids=list(range(E)))

    out = np.zeros((T, H), dtype=np.float32)
    for e in range(E):
        cnt = counts[e]
        if cnt:
            ye = res.results[e]["yT"][:, :cnt].T  # [cnt, H]
            out[idx[e]] += wts[e][:, None] * ye
    return out.reshape(x.shape)



# revision 16
# speedup vs baseline: 1.1489x; 1.0205x over previous
"""MoE MLP (Mixtral-style top-2 routing) on 8 Trainium2 NeuronCores.

Strategy: expert-parallel. The router (tiny: T x H x E) runs on host in fp32,
exactly mirroring the reference math. Tokens are grouped by expert on host;
core e runs a dense [C,H] -> silu/mul -> [C,H] MLP for expert e with bf16
matmuls (full PE rate + fast weight load) in a hand-scheduled raw-Bass
program. Host applies the top-k combine weights in a weighted scatter-add.

Device layout (per core, everything feature-on-partition, token-on-free):
  hT   [H=1024, C]   tokens for this expert, transposed (bf16)
  WgT  [H, F=4096]   gate weight, transposed (bf16)
  WuT  [H, F]        up weight, transposed (bf16)
  WdT  [F, H]        down weight, transposed (bf16)
  yT   [H, C]        output (unweighted expert output, transposed, f32)

Loop structure: passes over tokens (<=2816 tokens resident; a single pass in
practice); per pass loop over 8 F-blocks of 512 (weights double-buffered);
per block loop over 512-token ct tiles. Gate/up matmuls accumulate over H in
PSUM; ScalarE applies silu into the act tile; VectorE multiplies in-place by
the up projection; down matmuls accumulate the F-block in PSUM; VectorE
accumulates y in SBUF. The PE stream runs one ct-tile ahead (gate/up of
tile n+1 issued before down of tile n) to hide the silu/mul latency. y is
stored per-ht-tile as the last F-block's accumulations finish, so the store
overlaps the tail of compute.
"""

import ml_dtypes
import numpy as np
import concourse.bass as bass
import concourse.mybir as mybir
from concourse.bass_utils import run_bass_kernel_spmd

f32 = mybir.dt.float32
bf16 = mybir.dt.bfloat16
np_bf16 = ml_dtypes.bfloat16

B, S, H, F, E = 4, 2048, 1024, 4096, 8
KT = H // 128  # 8 k-tiles of the H contraction
NFB = 8  # F blocks
FBLK = F // NFB  # 512
FT_PER = FBLK // 128  # 4 f-tiles per block
HT = H // 128  # 8 output H tiles
CT_W = 512  # token tile width (moving dim N)


def _split_tiles(pass_size):
    """Split a pass into ct tiles: as few tiles as possible (<=512 each),
    near-equal widths, all multiples of 128 and >= 256."""
    k = -(-pass_size // CT_W)
    base = (pass_size // k) // 128 * 128
    widths = [base] * k
    rem = (pass_size - base * k) // 128
    for i in range(rem):
        widths[i] += 128
    assert sum(widths) == pass_size and all(256 <= w <= 512 for w in widths), widths
    return widths


def build_program(pass_sizes, repeat=1, probe=None):
    """Build the per-core Bass program for the given tuple of pass sizes
    (each a multiple of 256). `repeat` re-runs the whole computation that
    many times (same I/O) — benchmarking only. `probe` builds timing
    bisection variants (wrong results)."""
    pass_sizes = list(pass_sizes)
    C = sum(pass_sizes)
    pass_tok0 = [sum(pass_sizes[:p]) for p in range(len(pass_sizes))] * repeat
    pass_sizes = pass_sizes * repeat
    NP = len(pass_sizes)
    PSMAX = max(pass_sizes)
    tiles = [_split_tiles(ps) for ps in pass_sizes]
    NCT = [len(t) for t in tiles]

    # ctg enumeration: for p, for fb, for ct -> (p, fb, ct, width, offset)
    ctg_base = [0] * (NP + 1)
    for p in range(NP):
        ctg_base[p + 1] = ctg_base[p] + NFB * NCT[p]
    TOTAL_CT = ctg_base[NP]

    ctg_pfc = []
    for p in range(NP):
        offs = [sum(tiles[p][:i]) for i in range(NCT[p])]
        for fb in range(NFB):
            for ct in range(NCT[p]):
                ctg_pfc.append((p, fb, ct, tiles[p][ct], offs[ct]))

    def ctg_end_w(w):
        p, fb = divmod(w, NFB)
        return ctg_base[p] + (fb + 1) * NCT[p]

    hc_base = [sum(NCT[:p]) for p in range(NP)]

    NW = NP * NFB

    nc = bass.Bass()
    hT = nc.declare_dram_parameter("hT", [H, C], bf16, isOutput=False)
    wg = nc.declare_dram_parameter("WgT", [H, F], bf16, isOutput=False)
    wu = nc.declare_dram_parameter("WuT", [H, F], bf16, isOutput=False)
    wd = nc.declare_dram_parameter("WdT", [F, H], bf16, isOutput=False)
    yT = nc.declare_dram_parameter("yT", [H, C], f32, isOutput=True)

    hT_v = hT.rearrange("(k p) t -> p k t", p=128)  # [128, KT, C]
    wg_v = wg.rearrange("(k p) f -> p k f", p=128)  # [128, KT, F]
    wu_v = wu.rearrange("(k p) f -> p k f", p=128)
    wd_v = wd.rearrange("(q p) h -> p q h", p=128)  # [128, F//128, H]
    yT_v = yT.rearrange("(k p) t -> p k t", p=128)  # [128, HT, C]

    from contextlib import ExitStack

    with ExitStack() as ctx:
        en = ctx.enter_context
        h_sb = en(nc.sbuf_tensor("h_sb", [128, KT, PSMAX], bf16))
        h_pre = en(nc.sbuf_tensor("h_pre", [128, KT, CT_W], bf16))
        y_sb = en(nc.sbuf_tensor("y_sb", [128, HT, PSMAX], f32))
        wg_sb = en(nc.sbuf_tensor("wg_sb", [128, 2, KT, FBLK], bf16))
        wu_sb = en(nc.sbuf_tensor("wu_sb", [128, 2, KT, FBLK], bf16))
        wd_sb = en(nc.sbuf_tensor("wd_sb", [128, 2, FT_PER, H], bf16))
        act_sb = en(nc.sbuf_tensor("act_sb", [128, 2, FT_PER, CT_W], bf16))

        g_ps = [en(nc.psum_tensor(f"g_ps{i}", [128, CT_W], f32)) for i in range(2)]
        u_ps = [en(nc.psum_tensor(f"u_ps{i}", [128, CT_W], f32)) for i in range(2)]
        yp_ps = [en(nc.psum_tensor(f"yp_ps{i}", [128, CT_W], f32)) for i in range(4)]

        s_w = en(nc.semaphore())  # weight DMAs done (16/dma, 48/block)
        s_h = en(nc.semaphore())  # hT pass loads (16/tile, gp-issued)
        s_h0 = en(nc.semaphore())  # first h tile (sync-issued, 16)
        s_g = en(nc.semaphore())  # PE: gate groups done (1/gi)
        s_u = en(nc.semaphore())  # PE: up groups done (1/gi)
        s_silu = en(nc.semaphore())  # ACT: silu into act done (1/gi)
        s_mul = en(nc.semaphore())  # DVE: act *= up done (1/gi)
        s_down = en(nc.semaphore())  # PE: down groups done (1/di)
        s_yupd = en(nc.semaphore())  # DVE: y accum done (1/di)
        s_ydma = en(nc.semaphore())  # y store DMAs done (16/pass)

        block = en(nc.Block())

        # Single-pass fast path: y stores are split round-robin across the
        # sync/scalar/gpsimd queues so their ~0.7us DMA issue cost overlaps
        # the tail of compute instead of serializing on one queue.
        SINGLE = NP == 1 and probe is None

        def y_store_entries():
            p = NP - 1
            offs = [sum(tiles[p][:i]) for i in range(NCT[p])]
            out = []
            for ct in range(NCT[p]):
                ctg = ctg_base[p + 1] - NCT[p] + ct
                for ht in range(HT):
                    out.append((8 * ctg + ht + 1, ht, offs[ct], tiles[p][ct]))
            return out

        def emit_y_stores(eng, share, nshares=3):
            for i, (need, ht, coff, ctw) in enumerate(y_store_entries()):
                if i % nshares != share:
                    continue
                eng.wait_ge(s_yupd, need)
                eng.dma_start(
                    yT_v[:, ht, coff : coff + ctw], y_sb[:, ht, coff : coff + ctw]
                ).then_inc(s_ydma, 16)

        # ---------------- weight DMA stream (sync engine / HWDGE) --------
        # s_w thresholds: W=0 is split into ft-granular pieces so the PE can
        # start after the first ft column lands.
        # block 0 issues 9 DMAs (8 ft-granular wg/wu + wd) = 144 counts;
        # blocks >= 1 issue 3 DMAs (48 counts) each, order wg, wu, wd.
        def sw_need_gu(w, ft):
            if w == 0:
                return 32 * (ft + 1)
            return 144 + 48 * (w - 1) + 32

        def sw_need_down(w):
            return 144 + 48 * w

        @block.sync
        def _(sync):
            # first h tile via HWDGE: the sync queue's first DMA starts ~2us
            # earlier than gpsimd's SWDGE, shaving kernel startup
            w0 = tiles[0][0]
            sync.dma_start(h_sb[:, :, :w0], hT_v[:, :, 0:w0]).then_inc(s_h0, 16)
            for w in range(2 if probe == "wonce" else NW):
                p, fb = divmod(w, NFB)
                buf = w % 2
                if w >= 2:
                    if probe == "nodown":
                        sync.wait_ge(s_u, 4 * ctg_end_w(w - 2))
                    else:
                        sync.wait_ge(s_down, 8 * ctg_end_w(w - 2))
                fsl = slice(fb * FBLK, (fb + 1) * FBLK)
                qsl = slice(fb * FT_PER, (fb + 1) * FT_PER)
                if w == 0:
                    for ft in range(FT_PER):
                        f0 = fb * FBLK + ft * 128
                        sync.dma_start(
                            wg_sb[:, buf, :, ft * 128 : (ft + 1) * 128],
                            wg_v[:, :, f0 : f0 + 128],
                        ).then_inc(s_w, 16)
                        sync.dma_start(
                            wu_sb[:, buf, :, ft * 128 : (ft + 1) * 128],
                            wu_v[:, :, f0 : f0 + 128],
                        ).then_inc(s_w, 16)
                else:
                    sync.dma_start(wg_sb[:, buf], wg_v[:, :, fsl]).then_inc(s_w, 16)
                    sync.dma_start(wu_sb[:, buf], wu_v[:, :, fsl]).then_inc(s_w, 16)
                sync.dma_start(wd_sb[:, buf], wd_v[:, qsl, :]).then_inc(s_w, 16)
            if SINGLE:
                emit_y_stores(sync, 0)

        # ---------------- hT loads + y stores (gpsimd / SWDGE) -----------
        @block.gpsimd
        def _(gp):
            def load_h(p):
                # chunk 0 of pass p>=1 goes to the h_pre prefetch buffer,
                # issued as soon as the previous pass's first gu released it
                if p >= 1:
                    # h_pre is read at ct==0 of EVERY fb of pass p-1; free
                    # only after the last fb's gu of pass p-1
                    gp.wait_ge(s_u, 4 * (ctg_base[p - 1] + 7 * NCT[p - 1] + 1))
                    w0 = tiles[p][0]
                    tsl = slice(pass_tok0[p], pass_tok0[p] + w0)
                    gp.dma_start(h_pre[:, :, :w0], hT_v[:, :, tsl]).then_inc(s_h, 16)
                    gp.wait_ge(s_u, 4 * ctg_base[p])
                off = 0
                for i, wdt in enumerate(tiles[p]):
                    # pass 0 tile 0 is loaded by the sync engine (s_h0);
                    # pass >=1 tile 0 goes through h_pre above
                    if i == 0:
                        off += wdt
                        continue
                    tsl = slice(pass_tok0[p] + off, pass_tok0[p] + off + wdt)
                    gp.dma_start(
                        h_sb[:, :, off : off + wdt], hT_v[:, :, tsl]
                    ).then_inc(s_h, 16)
                    off += wdt

            def store_y(p):
                if probe == "nodown":
                    gp.wait_ge(s_mul, 4 * ctg_base[p + 1])
                elif probe in ("noyupd", "nosilu", "peonly"):
                    gp.wait_ge(s_down, 8 * ctg_base[p + 1])
                else:
                    gp.wait_ge(s_yupd, 8 * ctg_base[p + 1])
                tsl = slice(pass_tok0[p], pass_tok0[p] + pass_sizes[p])
                gp.dma_start(yT_v[:, :, tsl], y_sb[:, :, : pass_sizes[p]]).then_inc(
                    s_ydma, 16
                )

            if probe == "peonly":
                # init act with finite values (f32r memset fails ISA check)
                for b in range(2):
                    for ft in range(FT_PER):
                        gp.dma_start(act_sb[:, b, ft, :], hT_v[:, ft, 0:CT_W]).then_inc(
                            s_mul, 16
                        )
            load_h(0)
            for p in range(1, NP):
                load_h(p)
                if probe in ("nodown", "noyupd", "nosilu", "peonly"):
                    store_y(p - 1)
            if probe in ("nodown", "noyupd", "nosilu", "peonly"):
                store_y(NP - 1)
            if SINGLE:
                emit_y_stores(gp, 2)

        # ---------------- PE stream (one ct-tile lookahead) ----------------
        @block.tensor
        def _(te):
            def gu(ctg):
                p, fb, ct, ctw, coff = ctg_pfc[ctg]
                w = p * NFB + fb
                buf = w % 2
                if fb == 0:
                    if p == 0 and ct == 0:
                        te.wait_ge(s_h0, 16)
                    elif p == 0:
                        te.wait_ge(s_h, 16 * ct)
                    else:
                        te.wait_ge(s_h, 16 * (hc_base[p] + ct))
                if ct == 0 and w > 0:
                    te.wait_ge(
                        s_w,
                        min(sw_need_gu(w, 0), 96) if probe == "wonce" else sw_need_gu(w, 0),
                    )
                use_pre = p >= 1 and ct == 0
                csl = slice(coff, coff + ctw)
                for ft in range(FT_PER):
                    gi = ctg * 4 + ft
                    gb = gi % 2
                    if w == 0 and ct == 0 and probe != "wonce":
                        te.wait_ge(s_w, sw_need_gu(0, ft))
                    elif w == 0 and ct == 0 and ft == 0:
                        te.wait_ge(s_w, 64)
                    if gi >= 2 and probe not in ("nosilu", "peonly"):
                        te.wait_ge(s_silu, gi - 1)
                    for k in range(KT):
                        rhs = h_pre[:, k, :ctw] if use_pre else h_sb[:, k, csl]
                        mm = nc.tensor.matmul(
                            g_ps[gb][:, :ctw],
                            wg_sb[:, buf, k, ft * 128 : (ft + 1) * 128],
                            rhs,
                            start=(k == 0),
                            stop=(k == KT - 1),
                        )
                        if k == KT - 1:
                            mm.then_inc(s_g, 1)
                    if gi >= 2 and probe not in ("nosilu", "peonly"):
                        te.wait_ge(s_mul, gi - 1)
                    for k in range(KT):
                        rhs = h_pre[:, k, :ctw] if use_pre else h_sb[:, k, csl]
                        mm = nc.tensor.matmul(
                            u_ps[gb][:, :ctw],
                            wu_sb[:, buf, k, ft * 128 : (ft + 1) * 128],
                            rhs,
                            start=(k == 0),
                            stop=(k == KT - 1),
                        )
                        if k == KT - 1:
                            mm.then_inc(s_u, 1)

            def down(ctg):
                p, fb, ct, ctw, coff = ctg_pfc[ctg]
                ab = ctg % 2
                if ct == 0:
                    w = p * NFB + fb
                    te.wait_ge(
                        s_w,
                        min(sw_need_down(w), 144) if probe == "wonce" else sw_need_down(w),
                    )
                if probe == "peonly":
                    if ctg == 0:
                        te.wait_ge(s_mul, 128)  # act_sb init done
                elif probe != "nosilu":
                    te.wait_ge(s_mul, 4 * (ctg + 1))
                w = p * NFB + fb
                buf = w % 2
                for ht in range(HT):
                    di = ctg * 8 + ht
                    db = di % 4
                    if di >= 4 and probe not in ("noyupd", "nosilu", "peonly"):
                        te.wait_ge(s_yupd, di - 3)
                    for ft in range(FT_PER):
                        mm = nc.tensor.matmul(
                            yp_ps[db][:, :ctw],
                            wd_sb[:, buf, ft, ht * 128 : (ht + 1) * 128],
                            act_sb[:, ab, ft, :ctw],
                            start=(ft == 0),
                            stop=(ft == FT_PER - 1),
                        )
                        if ft == FT_PER - 1:
                            mm.then_inc(s_down, 1)

            gu(0)
            for ctg in range(TOTAL_CT):
                if ctg + 1 < TOTAL_CT:
                    same_pass = ctg_pfc[ctg + 1][0] == ctg_pfc[ctg][0]
                    if same_pass:
                        gu(ctg + 1)
                        if probe != "nodown":
                            down(ctg)
                    else:
                        if probe != "nodown":
                            down(ctg)
                        gu(ctg + 1)
                elif probe != "nodown":
                    down(ctg)

        # ---------------- ACT stream (silu into act tile) ------------------
        @block.scalar
        def _(sc):
            if probe == "peonly":
                sc.nop()
                return
            if probe == "nosilu":
                return

            def sc_store_y(p):
                # Chunked per-(ct, ht) stores: each chunk is final as soon as
                # the last F-block's yupd for it lands, so stores overlap the
                # tail of compute instead of waiting for the whole pass.
                offs = [sum(tiles[p][:i]) for i in range(NCT[p])]
                for ct in range(NCT[p]):
                    ctg = ctg_base[p + 1] - NCT[p] + ct
                    coff, ctw = offs[ct], tiles[p][ct]
                    tsl = slice(pass_tok0[p] + coff, pass_tok0[p] + coff + ctw)
                    for ht in range(HT):
                        sc.wait_ge(s_yupd, 8 * ctg + ht + 1)
                        sc.dma_start(
                            yT_v[:, ht, tsl], y_sb[:, ht, coff : coff + ctw]
                        ).then_inc(s_ydma, 16)

            for ctg in range(TOTAL_CT):
                p = ctg_pfc[ctg][0]
                if ctg > 0 and ctg_pfc[ctg - 1][0] != p:
                    sc_store_y(p - 1)
                ab = ctg % 2
                ctw = ctg_pfc[ctg][3]
                for ft in range(FT_PER):
                    gi = ctg * 4 + ft
                    gb = gi % 2
                    if ft == 0 and ctg >= 2:
                        # WAR on act_sb[ab]: down mms of ctg-2 done
                        if probe == "nodown":
                            sc.wait_ge(s_mul, 4 * (ctg - 1))
                        else:
                            sc.wait_ge(s_down, 8 * (ctg - 1))
                    sc.wait_ge(s_g, gi + 1)
                    nc.scalar.activation(
                        act_sb[:, ab, ft, :ctw],
                        g_ps[gb][:, :ctw],
                        mybir.ActivationFunctionType.Silu,
                    ).then_inc(s_silu, 1)
            if SINGLE:
                emit_y_stores(sc, 1)
            else:
                sc_store_y(NP - 1)

        # ---------------- DVE stream (mul + y accumulate) ------------------
        @block.vector
        def _(ve):
            if probe in ("nosilu", "peonly"):
                return

            def muls(ctg):
                ab = ctg % 2
                ctw = ctg_pfc[ctg][3]
                for ft in range(FT_PER):
                    gi = ctg * 4 + ft
                    gb = gi % 2
                    ve.wait_ge(s_silu, gi + 1)
                    ve.wait_ge(s_u, gi + 1)
                    nc.vector.tensor_mul(
                        act_sb[:, ab, ft, :ctw],
                        act_sb[:, ab, ft, :ctw],
                        u_ps[gb][:, :ctw],
                    ).then_inc(s_mul, 1)

            def yupd(ctg):
                if probe in ("nodown", "noyupd"):
                    return
                p, fb, ct, ctw, coff = ctg_pfc[ctg]
                csl = slice(coff, coff + ctw)
                for ht in range(HT):
                    di = ctg * 8 + ht
                    db = di % 4
                    ve.wait_ge(s_down, di + 1)
                    if fb == 0 and ct == 0 and ht == 0 and p > 0:
                        # all of the previous pass's chunked y stores done
                        ve.wait_ge(s_ydma, 16 * 8 * hc_base[p])
                    if fb == 0:
                        nc.vector.tensor_copy(
                            y_sb[:, ht, csl], yp_ps[db][:, :ctw]
                        ).then_inc(s_yupd, 1)
                    else:
                        nc.vector.tensor_add(
                            y_sb[:, ht, csl], y_sb[:, ht, csl], yp_ps[db][:, :ctw]
                        ).then_inc(s_yupd, 1)

            muls(0)
            for ctg in range(TOTAL_CT):
                # mirror the PE stream's emission order exactly, else the
                # crossing steps (down before gu) deadlock against us
                if ctg + 1 < TOTAL_CT:
                    same_pass = ctg_pfc[ctg + 1][0] == ctg_pfc[ctg][0]
                    if same_pass:
                        muls(ctg + 1)
                        yupd(ctg)
                    else:
                        yupd(ctg)
                        muls(ctg + 1)
                else:
                    yupd(ctg)

    return nc


# ----------------------------------------------------------------------------
# Host side
# ----------------------------------------------------------------------------


def _route(h, Wr, topk):
    """Exact fp32 replica of the reference router. Returns sel [T,k], w [T,k]."""
    logits = h @ Wr.T  # [T, E]
    logits = logits.astype(np.float32)
    m = logits.max(axis=-1, keepdims=True)
    e = np.exp(logits - m)
    p = e / e.sum(axis=-1, keepdims=True)
    sel = np.argsort(-p, axis=-1, kind="stable")[:, :topk]  # ties -> lower idx
    w = np.take_along_axis(p, sel, axis=-1)
    if topk != 1:
        w = w / w.sum(axis=-1, keepdims=True)
    return sel, w.astype(np.float32)


def _pass_sizes(C):
    # bf16 h + f32 y: 48 B/token/partition; 2816 tokens + weights fit SBUF
    n = -(-C // 2816)
    base = (C // n) // 128 * 128
    out = [base] * n
    rem = (C - base * n) // 128
    for i in range(rem):
        out[i] += 128
    assert sum(out) == C and all(ps <= 2816 for ps in out)
    return tuple(out)


def kernel(x, Wr, Wg, Wu, Wd, topk):
    topk = int(topk)
    x = np.asarray(x, dtype=np.float32)
    Wr = np.asarray(Wr, dtype=np.float32)
    Wg = np.asarray(Wg, dtype=np.float32)
    Wu = np.asarray(Wu, dtype=np.float32)
    Wd = np.asarray(Wd, dtype=np.float32)

    T = x.shape[0] * x.shape[1]
    h = np.ascontiguousarray(x.reshape(T, H))

    sel, w = _route(h, Wr, topk)

    idx = [None] * E
    wts = [None] * E
    for e in range(E):
        tok, kk = np.nonzero(sel == e)
        idx[e] = tok
        wts[e] = w[tok, kk]
    counts = [len(i) for i in idx]
    maxc = max(max(counts), 1)
    C = max(512, ((maxc + 127) // 128) * 128)

    nc = build_program(_pass_sizes(C))

    h16 = h.astype(np_bf16)
    hTfull = h16.T  # [H, T] view
    in_maps = []
    for e in range(E):
        cnt = counts[e]
        hTe = np.zeros((H, C), dtype=np_bf16)
        if cnt:
            hTe[:, :cnt] = hTfull[:, idx[e]]
        in_maps.append(
            {
                "hT": hTe,
                "WgT": np.ascontiguousarray(Wg[e].astype(np_bf16).T),  # [H, F]
                "WuT": np.ascontiguousarray(Wu[e].astype(np_bf16).T),  # [H, F]
                "WdT": np.ascontiguousarray(Wd[e].astype(np_bf16).T),  # [F, H]
            }
        )

    res = run_bass_kernel_spmd(nc, in_maps, core_ids=list(range(E)))

    out = np.zeros((T, H), dtype=np.float32)
    for e in range(E):
        cnt = counts[e]
        if cnt:
            ye = res.results[e]["yT"][:, :cnt].T  # [cnt, H]
            out[idx[e]] += wts[e][:, None] * ye
    return out.reshape(x.shape)

